# revision 14
# baseline (speedup 1.0000x reference)
"""Trainium2 Bass kernel for LocalPatternFilter.

Reference computation (per (b, h, c) row of length K=1024):
  1. gather window  g = X[b, c, pos[h] : pos[h]+K]
  2. fmax = max|g|;  w = g * hann / fmax
  3. acov = ifftshift(irfft(|rfft(w)|^2))   (= circular autocorrelation)

Implemented as folded DFT matmuls on the tensor engine in bf16.

Conjugate-symmetry fold (mm1): E_k = g[k]+g[1024-k], O_k = g[k]-g[1024-k]
(hann is symmetric, so it stays folded in the weights). Cos bins contract
only the 512 E samples, sin bins only the 512 O samples -> 36 PE
cycles/row instead of 64. The fold itself runs ON THE PE: each
transposed sample tile is the sum of two accumulating transposes -- the
forward gather block and a block of `grev` (a reversed copy of the
window, one strided DVE/GPSIMD copy per tile) with +identity for E tiles
and -identity for O tiles. hann[0]=0 frees the k=0 slot, which ends up
holding g[512] (coefficient (-1)^m for cos bins). Spectrum packing pairs
bins m and 512-m on the same partition:
  q0:C[p]  q1:C[128+p]  q2:C256|C[512-p]  q3:C[384-p]
  q4:S256|S[512-p]  q5:S[128+p]  q6:C512|S[p]  q7:S[384-p]
(C512 needs E samples, so chunk q6 contracts both halves: 8 matmuls.)

Parity fold (mm2): squares -> pair-adds Pf[m]=C^2+S^2 via (q0,q6),
(q2,q4), (q1,q5), (q3,q7); pe = Pf[m]+Pf[512-m], po = Pf[m]-Pf[512-m]
split the irfft into even/odd output column halves -> ~14 PE cycles/row
instead of 20. The polluted partition-0 lanes are repaired by rank-1
correction matmuls (single-lane DVE ops are ~7x slower than a 256-col
matmul, measured). acov[512] lands in the even half, so the old
Q-vector path is gone.

Stats: fmax = reduce_max|g| on DVE; 1/fmax^2 scales the single
PSUM->SBUF output copy per tile (on ACT, per-partition scale).

Gather: positions are compile-time constants. Head tiles 0-3 go as
regular HWDGE strided DMAs (constant-delta runs) so data flows ~1us in;
tiles 4-15 ride SWDGE indirect DMAs (the ~13us preamble hides behind
the head tiles). Weights are partition-major so each loads with one
large-descriptor DMA.

Output: acov[0:513] (even cols 0:257, odd cols 257:513), bf16;
de-interleave + mirror expansion + f32 cast on host.

Sharding: data-parallel over batch, 2 batches per core on 8 cores.
"""

import json

import numpy as np
import ml_dtypes

import concourse.bass as bass
import concourse.bass2jax as bass2jax
import concourse.bass_utils as bass_utils
import concourse.tile as tile
from concourse import mybir
from concourse.bass_utils import run_bass_kernel_spmd

_MAX_WAITS = 1


def _legalize_bir(bir_bytes):
    m = json.loads(bir_bytes)
    counter = [0]

    def fix_block(blk):
        insts = blk.get("instructions")
        if not isinstance(insts, list):
            return
        out = []
        for inst in insts:
            si = inst.get("sync_info") or {}
            waits = si.get("on_wait") or []
            if isinstance(inst.get("opcode"), str) and len(waits) > _MAX_WAITS:
                keep = waits[-_MAX_WAITS:]
                for w in waits[:-_MAX_WAITS]:
                    counter[0] += 1
                    out.append(
                        {
                            "debug": inst.get("debug", 0),
                            "engine": inst["engine"],
                            "ins": [],
                            "name": f"LGW-{counter[0]}-{inst['name']}",
                            "opcode": "EventSemaphore",
                            "outs": [],
                            "sync_info": {"on_update": [], "on_wait": [w]},
                        }
                    )
                si = dict(si)
                si["on_wait"] = keep
                inst = dict(inst)
                inst["sync_info"] = si
            out.append(inst)
        blk["instructions"] = out

    def walk(o):
        if isinstance(o, dict):
            if "instructions" in o:
                fix_block(o)
            for v in o.values():
                walk(v)
        elif isinstance(o, list):
            for v in o:
                walk(v)

    walk(m)
    return json.dumps(m).encode()


_orig_compile_bir_kernel = bass_utils.compile_bir_kernel


def _legalizing_compile_bir_kernel(bir_json, *args, **kwargs):
    if isinstance(bir_json, str):
        bir_json = bir_json.encode()
    return _orig_compile_bir_kernel(_legalize_bir(bir_json), *args, **kwargs)


if bass_utils.compile_bir_kernel is not _legalizing_compile_bir_kernel:
    bass_utils.compile_bir_kernel = _legalizing_compile_bir_kernel
    bass2jax.compile_bir_kernel = _legalizing_compile_bir_kernel

# positions = int32(jnp.linspace(0, L-2K-1, H)) for L=131072, K=1024, H=128
POS = [
    0, 1015, 2031, 3047, 4063, 5079, 6095, 7111, 8127, 9143, 10159, 11175,
    12191, 13207, 14223, 15238, 16254, 17270, 18286, 19302, 20318, 21334,
    22350, 23366, 24382, 25398, 26414, 27430, 28446, 29461, 30477, 31493,
    32509, 33525, 34541, 35557, 36573, 37589, 38605, 39621, 40637, 41653,
    42669, 43684, 44700, 45716, 46732, 47748, 48764, 49780, 50796, 51812,
    52828, 53844, 54860, 55876, 56892, 57907, 58923, 59939, 60955, 61971,
    62987, 64003, 65019, 66035, 67051, 68067, 69083, 70099, 71115, 72130,
    73146, 74162, 75178, 76194, 77210, 78226, 79242, 80258, 81274, 82290,
    83306, 84322, 85338, 86353, 87369, 88385, 89401, 90417, 91433, 92449,
    93465, 94481, 95497, 96513, 97529, 98545, 99561, 100576, 101592, 102608,
    103624, 104640, 105656, 106672, 107688, 108704, 109720, 110736, 111752,
    112768, 113784, 114799, 115815, 116831, 117847, 118863, 119879, 120895,
    121911, 122927, 123943, 124959, 125975, 126991, 128007, 129023,
]

N_CORES = 8
B_FULL, C_DIM, L_DIM = 16, 8, 131072
K_DIM, H_DIM = 1024, 128
B_LOC = B_FULL // N_CORES
NOUT = 513
NE, NO = 257, 256
HW_TILES = 10
F32 = mybir.dt.float32
BF16 = mybir.dt.bfloat16
I32 = mybir.dt.int32
NP_BF16 = ml_dtypes.bfloat16

_prog_cache = {}


def _tile_h0(jt):
    return (jt // 4 % 2) * 64 + (jt % 4) * 16


def _tile_runs(jt):
    h0 = _tile_h0(jt)
    pos = POS[h0 : h0 + 16]
    d = [pos[i + 1] - pos[i] for i in range(15)]
    runs = []
    s = 0
    for i in range(1, 15):
        if d[i] != d[i - 1]:
            runs.append((s, i - s + 1, d[s]))
            s = i + 1
    if s < 16:
        runs.append((s, 16 - s, d[s] if s < 15 else 1))
    return runs


def _cbin(q):
    p = np.arange(128)
    return [p, 128 + p, np.where(p == 0, 256, 512 - p), 384 - p][q]


def _sbin(q):
    p = np.arange(128)
    return [np.where(p == 0, 256, 512 - p), 128 + p, p, 384 - p][q - 4]


def make_constants():
    K = K_DIM
    h = 0.5 * (1.0 - np.cos(2.0 * np.pi * np.arange(K) / K))  # h[0] == 0

    w1e = np.zeros((128, 4, 640))
    w1o = np.zeros((128, 4, 512))
    for t in range(4):
        for p in range(128):
            k = 128 * t + p
            for cq in range(4):
                m = _cbin(cq)
                if k == 0:
                    w1e[p, t, 128 * cq : 128 * (cq + 1)] = (-1.0) ** m
                else:
                    w1e[p, t, 128 * cq : 128 * (cq + 1)] = h[k] * np.cos(
                        2 * np.pi * k * m / K
                    )
            w1e[p, t, 512] = 1.0 if k == 0 else h[k] * ((-1.0) ** k)
    for t in range(4):
        for p in range(128):
            o = 128 * t + p
            if o == 0:
                continue
            for q in range(4, 8):
                m = _sbin(q)
                vals = h[o] * np.sin(2 * np.pi * o * m / K)
                if q == 6:
                    vals = vals.copy()
                    vals[0] = 0.0
                w1o[p, t, 128 * (q - 4) : 128 * (q - 3)] = vals
    w1e = w1e.astype(np.float32).astype(NP_BF16)
    w1o = w1o.astype(np.float32).astype(NP_BF16)

    e = np.arange(NE)[None, :]
    pc = np.arange(128)[:, None]
    we0 = 2.0 * np.cos(2 * np.pi * pc * e / 512) / K
    we0[0, :] = 1.0 / K
    we1 = 2.0 * np.cos(2 * np.pi * (128 + pc) * e / 512) / K
    wev = np.zeros((128, 2, 260))
    wev[:, 0, 0:NE] = we0
    wev[:, 1, 0:NE] = we1
    d_odd = 2 * np.arange(NO)[None, :] + 1
    wod = np.zeros((128, 2, NO))
    wod[:, 0] = 2.0 * np.cos(2 * np.pi * pc * d_odd / K) / K
    wod[0, 0] = 1.0 / K
    wod[:, 1] = 2.0 * np.cos(2 * np.pi * (128 + pc) * d_odd / K) / K
    # lane-0 correction matrices (only row 0 nonzero; applied as full
    # 128-partition matmuls against the pa1 / sq6 tiles, whose lane 0
    # holds Pf256 / Pf512): [0:260) even wpf - we0row0, [260:520) odd
    # +1/K, [520:780) odd -2/K
    wcor = np.zeros((128, 780))
    wcor[0, 0:NE] = 2.0 * ((-1.0) ** np.arange(NE)) / K - 1.0 / K
    wcor[0, 260 : 260 + NO] = 1.0 / K
    wcor[0, 520 : 520 + NO] = -2.0 / K
    wev = wev.astype(np.float32).astype(NP_BF16)
    wod = wod.astype(np.float32).astype(NP_BF16)
    wcor = wcor.astype(np.float32).astype(NP_BF16)

    ident = np.eye(128, dtype=np.float32).astype(NP_BF16)
    negid = (-np.eye(128, dtype=np.float32)).astype(NP_BF16)

    ntl = 16 - HW_TILES
    gidx = np.zeros((ntl, 128), dtype=np.int32)
    for j, jt in enumerate(range(HW_TILES, 16)):
        b = jt // 8
        h0 = _tile_h0(jt)
        for hh in range(16):
            for c in range(C_DIM):
                gidx[j, hh * 8 + c] = (
                    b * C_DIM * L_DIM + c * L_DIM + POS[h0 + hh]
                )
    gidx_t = np.ascontiguousarray(gidx.T)
    return {"w1e": w1e, "w1o": w1o, "wev": wev, "wod": wod, "wcor": wcor,
            "ident": ident, "negid": negid, "gidx": gidx_t}


def build_program():
    nc = bass.Bass("TRN2", target_bir_lowering=False, debug=False,
                   num_swdge_queues=4)
    x = nc.dram_tensor("x", [B_LOC, C_DIM, L_DIM], BF16,
                       kind="ExternalInput").ap()
    w1e = nc.dram_tensor("w1e", [128, 4, 640], BF16,
                         kind="ExternalInput").ap()
    w1o = nc.dram_tensor("w1o", [128, 4, 512], BF16,
                         kind="ExternalInput").ap()
    wev = nc.dram_tensor("wev", [128, 2, 260], BF16,
                         kind="ExternalInput").ap()
    wod = nc.dram_tensor("wod", [128, 2, 256], BF16,
                         kind="ExternalInput").ap()
    wcor = nc.dram_tensor("wcor", [128, 780], BF16,
                          kind="ExternalInput").ap()
    ident = nc.dram_tensor("ident", [128, 128], BF16,
                           kind="ExternalInput").ap()
    negid = nc.dram_tensor("negid", [128, 128], BF16,
                           kind="ExternalInput").ap()
    gidx = nc.dram_tensor("gidx", [128, 16 - HW_TILES], I32,
                          kind="ExternalInput").ap()
    y = nc.dram_tensor(
        "y", [B_LOC, H_DIM, C_DIM, NOUT], BF16, kind="ExternalOutput"
    ).ap()

    with tile.TileContext(nc) as tc:
        with (
            tc.tile_pool(name="singles", bufs=1) as singles,
            tc.tile_pool(name="gather", bufs=16) as gpool,
            tc.tile_pool(name="grev", bufs=16) as rpool,
            tc.tile_pool(name="wt", bufs=2) as wtpool,
            tc.tile_pool(name="sq", bufs=2) as sqpool,
            tc.tile_pool(name="pa", bufs=2) as papool,
            tc.tile_pool(name="pp", bufs=2) as pppool,
            tc.tile_pool(name="yy", bufs=4) as ypool,
            tc.tile_pool(name="small", bufs=48) as smallpool,
            tc.tile_pool(name="tp_ps", bufs=2, space="PSUM") as tp_ps_pool,
            tc.tile_pool(name="mm1_ps", bufs=2, space="PSUM") as mm1_ps_pool,
            tc.tile_pool(name="mm2_ps", bufs=2, space="PSUM") as mm2_ps_pool,
        ):
            id_sb = singles.tile([128, 128], BF16)
            nc.sync.dma_start(out=id_sb, in_=ident)
            nid_sb = singles.tile([128, 128], BF16)
            nc.sync.dma_start(out=nid_sb, in_=negid)

            gts = [None] * 16
            for _jt in range(16):
                gts[_jt] = gpool.tile([128, 1026], BF16, tag="gt",
                                      name=f"gt{_jt}")

            def emit_gather_runs(jt, eng):
                gt = gts[jt]
                b = jt // 8
                h0 = _tile_h0(jt)
                for (s, r, delta) in _tile_runs(jt):
                    src = bass.AP(
                        tensor=x.tensor,
                        offset=b * C_DIM * L_DIM + POS[h0 + s],
                        ap=[[delta, r], [L_DIM, C_DIM], [1, K_DIM]],
                    )
                    dst = bass.AP(
                        tensor=gt.tensor,
                        offset=gt.offset + 8 * s * 1026,
                        ap=[[1026, 8 * r], [1, K_DIM]],
                    )
                    eng.dma_start(out=dst, in_=src)

            # HWDGE head gathers, both rings: SP (mostly idle engine)
            # takes the even head tiles + weights; ACT takes tiles 1,3
            # now and 5,7,9 deferred into the first groups' compute
            # slots (its FIFO also runs squares/copies).
            for _jt in (0, 2):
                emit_gather_runs(_jt, nc.sync)
            for _jt in (1, 3, 5, 7, 9):
                emit_gather_runs(_jt, nc.scalar)

            # weights on the SP ring (one large-descriptor DMA each)
            w1e_sb = singles.tile([128, 4, 640], BF16)
            nc.sync.dma_start(out=w1e_sb, in_=w1e)
            w1o_sb = singles.tile([128, 4, 512], BF16)
            nc.sync.dma_start(out=w1o_sb, in_=w1o)
            wev_sb = singles.tile([128, 2, 260], BF16)
            nc.sync.dma_start(out=wev_sb, in_=wev)
            for _jt in (4, 6, 8):
                emit_gather_runs(_jt, nc.sync)
            wod_sb = singles.tile([128, 2, 256], BF16)
            nc.sync.dma_start(out=wod_sb, in_=wod)
            wcor_sb = singles.tile([128, 780], BF16)
            nc.sync.dma_start(out=wcor_sb, in_=wcor)
            gidx_sb = singles.tile([128, 16 - HW_TILES], I32)
            nc.sync.dma_start(out=gidx_sb, in_=gidx)

            # SWDGE gathers for tiles 10..15
            x_flat = x.rearrange("b c l -> (b c) l")
            for jt in range(HW_TILES, 16):
                gt = gts[jt]
                j = jt - HW_TILES
                gd = nc.gpsimd.indirect_dma_start(
                    out=gt[:, 0:K_DIM],
                    out_offset=None,
                    in_=x_flat,
                    in_offset=bass.IndirectOffsetOnAxis(
                        ap=gidx_sb[:, j : j + 1], axis=1
                    ),
                )
                qi = j % 4
                if qi:
                    gd.ins.queue = f"qPoolDynamic{qi}"

            # PE warmup: keep the HAM clock-gate fed until real transposes
            for w in range(11):
                tpw = tp_ps_pool.tile([128, 512], BF16, tag="tp")
                for i in range(4):
                    nc.tensor.transpose(
                        tpw[:, 128 * i : 128 * (i + 1)], id_sb, id_sb
                    )
            for w in range(4):
                tpw = tp_ps_pool.tile([128, 512], BF16, tag="tp")
                for i in range(8):
                    nc.tensor.transpose(
                        tpw[:, 64 * i : 64 * (i + 1)], id_sb, id_sb[:, 0:64]
                    )

            inv2s = [None] * 16
            grevs = [None] * 16

            def tile_prep(jt, rev_eng):
                gt = gts[jt]
                fm = smallpool.tile([128, 1], F32, tag="fm")
                nc.vector.reduce_max(
                    out=fm, in_=gt[:, 0:K_DIM],
                    axis=mybir.AxisListType.X,
                    apply_absolute_value=True,
                )
                inv = smallpool.tile([128, 1], F32, tag="inv")
                nc.vector.reciprocal(out=inv, in_=fm)
                inv2 = smallpool.tile([128, 1], F32, tag="inv2")
                nc.vector.tensor_mul(inv2, inv, inv)
                inv2s[jt] = inv2
                # k=0 slot dies (hann[0]=0) -> zero it so the E fold puts
                # g[512] there (via grev[0] = col 1024); O slot 0 = -g512
                # (zero-weighted)
                nc.vector.memset(gt[:, 0:1], 0)
                nc.vector.tensor_copy(gt[:, 1024:1025], gt[:, 512:513])
                gr = rpool.tile([128, 512], BF16, tag="gr")
                grevs[jt] = gr
                rev = bass.AP(
                    tensor=gt.tensor,
                    offset=gt.offset + 1024,
                    ap=[list(gt.ap[0]), [-1, 512]],
                )
                eng = nc.vector if rev_eng == "v" else nc.gpsimd
                eng.tensor_copy(gr, rev)

            def mm1_q(wt_t, sq, q, rows):
                mq = mm1_ps_pool.tile([128, 512], F32, tag="mm1")
                if q < 4:
                    ops = [(w1e_sb, 128 * q, t, t) for t in range(4)]
                elif q != 6:
                    ops = [(w1o_sb, 128 * (q - 4), t, 4 + t)
                           for t in range(4)]
                else:
                    ops = [(w1o_sb, 256, t, 4 + t) for t in range(4)]
                    ops += [(w1e_sb, 512, t, t) for t in range(4)]
                n = len(ops)
                for i, (wsb, c0, t, slot) in enumerate(ops):
                    nc.tensor.matmul(
                        mq[:, 0:rows],
                        wsb[:, t, c0 : c0 + 128],
                        wt_t[:, slot, :],
                        start=(i == 0),
                        stop=(i == n - 1),
                    )
                nc.scalar.square(sq[:, q, :], mq[:, 0:rows])

            def y_out(yp, jt):
                ysb = ypool.tile([128, 516], BF16, tag="y")
                nc.scalar.mul(ysb[:, 0:NOUT], yp[:, 0:NOUT], inv2s[jt])
                b = jt // 8
                hs = _tile_h0(jt)
                dst = y[b, hs : hs + 16].rearrange("h c n -> (h c) n")
                nc.sync.dma_start(out=dst, in_=ysb[:, 0:NOUT])

            def mm2_head(yp, pp, pa, sq, r0):
                # ONE accumulation group per yp tile: start only on the
                # first matmul, stop only on the last (interleaved
                # start/stop groups within one PSUM bank corrupt each
                # other, measured). The odd half's first writer hits
                # virgin has_written=0 elements, so start=False still
                # overwrites there. skip_group_check: bass pairs
                # start/stop per column-range, the hardware per bank.
                nc.tensor.matmul(
                    yp[:, 0:NE], pp[:, 0, r0 : r0 + 128], wev_sb[:, 0, 0:NE],
                    start=True, stop=False, skip_group_check=True,
                )
                nc.tensor.matmul(
                    yp[:, 0:NE], pa[:, 1, r0 : r0 + 128],
                    wcor_sb[:, 0:NE], start=False, stop=False,
                    skip_group_check=True,
                )
                nc.tensor.matmul(
                    yp[:, NE : NE + 255], pp[:, 1, r0 : r0 + 128],
                    wod_sb[:, 0, 0:255], start=False, stop=False,
                    skip_group_check=True,
                )
                nc.tensor.matmul(
                    yp[:, NE : NE + 255], pa[:, 1, r0 : r0 + 128],
                    wcor_sb[:, 260 : 260 + 255], start=False, stop=False,
                    skip_group_check=True,
                )
                nc.tensor.matmul(
                    yp[:, NE : NE + 255], sq[:, 6, r0 : r0 + 128],
                    wcor_sb[:, 520 : 520 + 255], start=False, stop=False,
                    skip_group_check=True,
                )

            def mm2_tail2(yp, pp, pa, sq, r0):
                nc.tensor.matmul(
                    yp[:, 0:NE], pp[:, 2, r0 : r0 + 128], wev_sb[:, 1, 0:NE],
                    start=False, stop=False, skip_group_check=True,
                )
                nc.tensor.matmul(
                    yp[:, NE : NE + 255], pp[:, 3, r0 : r0 + 128],
                    wod_sb[:, 1, 0:255], start=False, stop=True,
                    skip_group_check=True,
                )
                # odd e=255 (device col 512) lives in the next PSUM bank;
                # it gets its own properly-bracketed 1-col group
                nc.tensor.matmul(
                    yp[:, 512:513], pp[:, 1, r0 : r0 + 128],
                    wod_sb[:, 0, 255:256], start=True, stop=False,
                )
                nc.tensor.matmul(
                    yp[:, 512:513], pa[:, 1, r0 : r0 + 128],
                    wcor_sb[:, 515:516], start=False, stop=False,
                )
                nc.tensor.matmul(
                    yp[:, 512:513], sq[:, 6, r0 : r0 + 128],
                    wcor_sb[:, 775:776], start=False, stop=False,
                )
                nc.tensor.matmul(
                    yp[:, 512:513], pp[:, 3, r0 : r0 + 128],
                    wod_sb[:, 1, 255:256], start=False, stop=True,
                )

            def mm2_rt(tile0, pp, pa, sq, rt):
                jt = tile0 + rt
                yp = mm2_ps_pool.tile([128, 516], F32, tag="mm2")
                r0 = 128 * rt
                mm2_head(yp, pp, pa, sq, r0)
                mm2_tail2(yp, pp, pa, sq, r0)
                y_out(yp, jt)

            GROUPS = [(0, 2), (2, 2), (4, 4), (8, 4), (12, 2), (14, 2)]
            PREP = [(2, 3), (4, 5, 6, 7), (8, 9, 10, 11), (12, 13, 14, 15),
                    (), ()]
            REV_ENG = ["v"] * 10 + ["g"] * 6

            prev = None            # (tile0, pp, pa, sq, ntiles)
            tail_yps = []
            CHUNK_ORDER = (0, 6, 2, 4, 1, 5, 3, 7)
            deferred = []

            def emit_deferred():
                if deferred:
                    djt, (rs, rr, dd) = deferred.pop(0)
                    src = bass.AP(
                        tensor=x.tensor,
                        offset=(djt // 8) * C_DIM * L_DIM
                        + POS[_tile_h0(djt) + rs],
                        ap=[[dd, rr], [L_DIM, C_DIM], [1, K_DIM]],
                    )
                    dst = bass.AP(
                        tensor=gts[djt].tensor,
                        offset=gts[djt].offset + 8 * rs * 1026,
                        ap=[[1026, 8 * rr], [1, K_DIM]],
                    )
                    nc.scalar.dma_start(out=dst, in_=src)

            tile_prep(0, REV_ENG[0])
            tile_prep(1, REV_ENG[1])

            for gi, (tile0, nt) in enumerate(GROUPS):
                last = gi == len(GROUPS) - 1
                rows = 128 * nt
                wt_t = wtpool.tile([128, 8, 512], BF16, tag="wt")
                wt_t = wt_t[:, :, 0:rows]
                sq = sqpool.tile([128, 8, 512], BF16, tag="sq")
                sq = sq[:, :, 0:rows]
                pa = papool.tile([128, 4, 512], BF16, tag="pa")
                pa = pa[:, :, 0:rows]
                pp = pppool.tile([128, 4, 512], BF16, tag="pp")
                pp = pp[:, :, 0:rows]
                prt = 0 if prev is None else prev[4]
                # transposes: each sample chunk is a pair of accumulating
                # REGULAR matmuls against +/-identity (the xbar transpose
                # path ignores PSUM accumulation, measured) -- fwd gather
                # block + grev block realize the E/O fold on the PE, summed
                # in f32 PSUM, then one cast-copy per chunk to bf16 SBUF.
                for slot in range(8):
                    tp = tp_ps_pool.tile([128, 512], F32, tag="tp")
                    blk = slot if slot < 4 else slot - 4
                    rid = id_sb if slot < 4 else nid_sb
                    for i in range(nt):
                        o = 128 * i
                        nc.tensor.matmul(
                            tp[:, o : o + 128],
                            gts[tile0 + i][:, 128 * blk : 128 * blk + 128],
                            id_sb, start=True, stop=False,
                        )
                        nc.tensor.matmul(
                            tp[:, o : o + 128],
                            grevs[tile0 + i][:, 128 * blk : 128 * blk + 128],
                            rid, start=False, stop=True,
                        )
                    if slot % 2 == 0:
                        nc.vector.tensor_copy(wt_t[:, slot, :],
                                              tp[:, 0:rows])
                    else:
                        nc.scalar.copy(out=wt_t[:, slot, :],
                                       in_=tp[:, 0:rows])
                    if last:
                        if slot == 2 and prt > 0:
                            mm2_rt(prev[0], prev[1], prev[2], prev[3], 0)
                        if slot == 5 and prt > 0:
                            mm2_rt(prev[0], prev[1], prev[2], prev[3], 1)
                    elif slot == 5 and prt > 0:
                        mm2_rt(prev[0], prev[1], prev[2], prev[3], 0)
                slist = PREP[gi]
                for step, q in enumerate(CHUNK_ORDER):
                    mm1_q(wt_t, sq, q, rows)
                    if gi < 2:
                        emit_deferred()
                    if step < len(slist):
                        tile_prep(slist[step], REV_ENG[slist[step]])
                    if step == 1:
                        nc.vector.tensor_add(pa[:, 0, :], sq[:, 0, :],
                                             sq[:, 6, :])
                    elif step == 3:
                        nc.vector.tensor_add(pa[:, 1, :], sq[:, 2, :],
                                             sq[:, 4, :])
                        nc.vector.tensor_add(pp[:, 0, :], pa[:, 0, :],
                                             pa[:, 1, :])
                        nc.vector.tensor_sub(pp[:, 1, :], pa[:, 0, :],
                                             pa[:, 1, :])
                    elif step == 5:
                        nc.vector.tensor_add(pa[:, 2, :], sq[:, 1, :],
                                             sq[:, 5, :])
                    elif step == 7:
                        nc.vector.tensor_add(pa[:, 3, :], sq[:, 3, :],
                                             sq[:, 7, :])
                        nc.vector.tensor_add(pp[:, 2, :], pa[:, 2, :],
                                             pa[:, 3, :])
                        nc.vector.tensor_sub(pp[:, 3, :], pa[:, 2, :],
                                             pa[:, 3, :])
                    if step == 1 and prt > 2:
                        mm2_rt(prev[0], prev[1], prev[2], prev[3], 1)
                    if step == 3 and prt > 2:
                        mm2_rt(prev[0], prev[1], prev[2], prev[3], 2)
                    if step == 5 and prt > 2:
                        mm2_rt(prev[0], prev[1], prev[2], prev[3], 3)
                    if step == 3 and not last and 0 < prt <= 2:
                        mm2_rt(prev[0], prev[1], prev[2], prev[3], 1)
                    if last:
                        if step == 3:
                            tail_yps = [
                                mm2_ps_pool.tile(
                                    [128, 516], F32, tag="mm2",
                                    name=f"typ{_rt}",
                                )
                                for _rt in range(nt)
                            ]
                            for rt, yp in enumerate(tail_yps):
                                mm2_head(yp, pp, pa, sq, 128 * rt)
                        if step == 7:
                            for rt, yp in enumerate(tail_yps):
                                mm2_tail2(yp, pp, pa, sq, 128 * rt)
                                y_out(yp, tile0 + rt)
                prev = (tile0, pp, pa, sq, nt)
    return nc


def get_program():
    if "nc" not in _prog_cache:
        _prog_cache["nc"] = build_program()
        _prog_cache["consts"] = make_constants()
    return _prog_cache["nc"], _prog_cache["consts"]


def kernel(X, kernel_size=None, out_channels=None, _trace=False):
    X = np.ascontiguousarray(
        np.asarray(X, dtype=np.float32).astype(NP_BF16)
    )
    assert X.shape == (B_FULL, C_DIM, L_DIM)
    nc, consts = get_program()
    in_maps = []
    for c in range(N_CORES):
        m = {"x": X[c * B_LOC : (c + 1) * B_LOC]}
        m.update(consts)
        in_maps.append(m)
    res = run_bass_kernel_spmd(
        nc, in_maps, core_ids=list(range(N_CORES)), trace=_trace
    )
    raw = np.concatenate(
        [np.asarray(r["y"]).astype(np.float32) for r in res.results], axis=0
    )  # (B, H, C, 513): cols 0:257 = acov[0::2], 257:513 = acov[1::2]
    acov = np.empty_like(raw)
    acov[..., 0::2] = raw[..., 0:NE]
    acov[..., 1::2] = raw[..., NE:NOUT]
    out = np.concatenate(
        [acov[..., 512:0:-1], acov[..., 0:512]], axis=-1
    )
    if _trace:
        return out, res
    return out


# revision 15
# speedup vs baseline: 1.0307x; 1.0307x over previous
"""Trainium2 Bass kernel for LocalPatternFilter.

Reference computation (per (b, h, c) row of length K=1024):
  1. gather window  g = X[b, c, pos[h] : pos[h]+K]
  2. fmax = max|g|;  w = g * hann / fmax
  3. acov = ifftshift(irfft(|rfft(w)|^2))   (= circular autocorrelation)

Implemented as folded DFT matmuls on the tensor engine in bf16.

Conjugate-symmetry fold (mm1): E_k = g[k]+g[1024-k], O_k = g[k]-g[1024-k]
(hann is symmetric, so it stays folded in the weights). Cos bins contract
only the 512 E samples, sin bins only the 512 O samples -> 36 PE
cycles/row instead of 64. The fold itself runs ON THE PE: each
transposed sample tile is the sum of two accumulating transposes -- the
forward gather block and a block of `grev` (a reversed copy of the
window, one strided DVE/GPSIMD copy per tile) with +identity for E tiles
and -identity for O tiles. hann[0]=0 frees the k=0 slot, which ends up
holding g[512] (coefficient (-1)^m for cos bins). Spectrum packing pairs
bins m and 512-m on the same partition:
  q0:C[p]  q1:C[128+p]  q2:C256|C[512-p]  q3:C[384-p]
  q4:S256|S[512-p]  q5:S[128+p]  q6:C512|S[p]  q7:S[384-p]
(C512 needs E samples, so chunk q6 contracts both halves: 8 matmuls.)

Parity fold (mm2): squares -> pair-adds Pf[m]=C^2+S^2 via (q0,q6),
(q2,q4), (q1,q5), (q3,q7); pe = Pf[m]+Pf[512-m], po = Pf[m]-Pf[512-m]
split the irfft into even/odd output column halves -> ~14 PE cycles/row
instead of 20. The polluted partition-0 lanes are repaired by rank-1
correction matmuls (single-lane DVE ops are ~7x slower than a 256-col
matmul, measured). acov[512] lands in the even half, so the old
Q-vector path is gone.

Stats: fmax = reduce_max|g| on DVE; 1/fmax^2 scales the single
PSUM->SBUF output copy per tile (on ACT, per-partition scale).

Gather: positions are compile-time constants. Head tiles 0-3 go as
regular HWDGE strided DMAs (constant-delta runs) so data flows ~1us in;
tiles 4-15 ride SWDGE indirect DMAs (the ~13us preamble hides behind
the head tiles). Weights are partition-major so each loads with one
large-descriptor DMA.

Output: acov[0:513] (even cols 0:257, odd cols 257:513), bf16;
de-interleave + mirror expansion + f32 cast on host.

Sharding: data-parallel over batch, 2 batches per core on 8 cores.
"""

import json

import numpy as np
import ml_dtypes

import concourse.bass as bass
import concourse.bass2jax as bass2jax
import concourse.bass_utils as bass_utils
import concourse.tile as tile
from concourse import mybir
from concourse.bass_utils import run_bass_kernel_spmd

_MAX_WAITS = 1


def _legalize_bir(bir_bytes):
    m = json.loads(bir_bytes)
    counter = [0]

    def fix_block(blk):
        insts = blk.get("instructions")
        if not isinstance(insts, list):
            return
        out = []
        for inst in insts:
            si = inst.get("sync_info") or {}
            waits = si.get("on_wait") or []
            if isinstance(inst.get("opcode"), str) and len(waits) > _MAX_WAITS:
                keep = waits[-_MAX_WAITS:]
                for w in waits[:-_MAX_WAITS]:
                    counter[0] += 1
                    out.append(
                        {
                            "debug": inst.get("debug", 0),
                            "engine": inst["engine"],
                            "ins": [],
                            "name": f"LGW-{counter[0]}-{inst['name']}",
                            "opcode": "EventSemaphore",
                            "outs": [],
                            "sync_info": {"on_update": [], "on_wait": [w]},
                        }
                    )
                si = dict(si)
                si["on_wait"] = keep
                inst = dict(inst)
                inst["sync_info"] = si
            out.append(inst)
        blk["instructions"] = out

    def walk(o):
        if isinstance(o, dict):
            if "instructions" in o:
                fix_block(o)
            for v in o.values():
                walk(v)
        elif isinstance(o, list):
            for v in o:
                walk(v)

    walk(m)
    return json.dumps(m).encode()


_orig_compile_bir_kernel = bass_utils.compile_bir_kernel


def _legalizing_compile_bir_kernel(bir_json, *args, **kwargs):
    if isinstance(bir_json, str):
        bir_json = bir_json.encode()
    return _orig_compile_bir_kernel(_legalize_bir(bir_json), *args, **kwargs)


if bass_utils.compile_bir_kernel is not _legalizing_compile_bir_kernel:
    bass_utils.compile_bir_kernel = _legalizing_compile_bir_kernel
    bass2jax.compile_bir_kernel = _legalizing_compile_bir_kernel

# positions = int32(jnp.linspace(0, L-2K-1, H)) for L=131072, K=1024, H=128
POS = [
    0, 1015, 2031, 3047, 4063, 5079, 6095, 7111, 8127, 9143, 10159, 11175,
    12191, 13207, 14223, 15238, 16254, 17270, 18286, 19302, 20318, 21334,
    22350, 23366, 24382, 25398, 26414, 27430, 28446, 29461, 30477, 31493,
    32509, 33525, 34541, 35557, 36573, 37589, 38605, 39621, 40637, 41653,
    42669, 43684, 44700, 45716, 46732, 47748, 48764, 49780, 50796, 51812,
    52828, 53844, 54860, 55876, 56892, 57907, 58923, 59939, 60955, 61971,
    62987, 64003, 65019, 66035, 67051, 68067, 69083, 70099, 71115, 72130,
    73146, 74162, 75178, 76194, 77210, 78226, 79242, 80258, 81274, 82290,
    83306, 84322, 85338, 86353, 87369, 88385, 89401, 90417, 91433, 92449,
    93465, 94481, 95497, 96513, 97529, 98545, 99561, 100576, 101592, 102608,
    103624, 104640, 105656, 106672, 107688, 108704, 109720, 110736, 111752,
    112768, 113784, 114799, 115815, 116831, 117847, 118863, 119879, 120895,
    121911, 122927, 123943, 124959, 125975, 126991, 128007, 129023,
]

N_CORES = 8
B_FULL, C_DIM, L_DIM = 16, 8, 131072
K_DIM, H_DIM = 1024, 128
B_LOC = B_FULL // N_CORES
NOUT = 513
NE, NO = 257, 256
HW_TILES = 4
F32 = mybir.dt.float32
BF16 = mybir.dt.bfloat16
I32 = mybir.dt.int32
NP_BF16 = ml_dtypes.bfloat16

_prog_cache = {}


def _tile_h0(jt):
    return (jt // 4 % 2) * 64 + (jt % 4) * 16


def _tile_runs(jt):
    h0 = _tile_h0(jt)
    pos = POS[h0 : h0 + 16]
    d = [pos[i + 1] - pos[i] for i in range(15)]
    runs = []
    s = 0
    for i in range(1, 15):
        if d[i] != d[i - 1]:
            runs.append((s, i - s + 1, d[s]))
            s = i + 1
    if s < 16:
        runs.append((s, 16 - s, d[s] if s < 15 else 1))
    return runs


def _cbin(q):
    p = np.arange(128)
    return [p, 128 + p, np.where(p == 0, 256, 512 - p), 384 - p][q]


def _sbin(q):
    p = np.arange(128)
    return [np.where(p == 0, 256, 512 - p), 128 + p, p, 384 - p][q - 4]


def make_constants():
    K = K_DIM
    h = 0.5 * (1.0 - np.cos(2.0 * np.pi * np.arange(K) / K))  # h[0] == 0

    w1e = np.zeros((128, 4, 640))
    w1o = np.zeros((128, 4, 512))
    for t in range(4):
        for p in range(128):
            k = 128 * t + p
            for cq in range(4):
                m = _cbin(cq)
                if k == 0:
                    w1e[p, t, 128 * cq : 128 * (cq + 1)] = (-1.0) ** m
                else:
                    w1e[p, t, 128 * cq : 128 * (cq + 1)] = h[k] * np.cos(
                        2 * np.pi * k * m / K
                    )
            w1e[p, t, 512] = 1.0 if k == 0 else h[k] * ((-1.0) ** k)
    for t in range(4):
        for p in range(128):
            o = 128 * t + p
            if o == 0:
                continue
            for q in range(4, 8):
                m = _sbin(q)
                vals = h[o] * np.sin(2 * np.pi * o * m / K)
                if q == 6:
                    vals = vals.copy()
                    vals[0] = 0.0
                w1o[p, t, 128 * (q - 4) : 128 * (q - 3)] = vals
    w1e = w1e.astype(np.float32).astype(NP_BF16)
    w1o = w1o.astype(np.float32).astype(NP_BF16)

    e = np.arange(NE)[None, :]
    pc = np.arange(128)[:, None]
    we0 = 2.0 * np.cos(2 * np.pi * pc * e / 512) / K
    we0[0, :] = 1.0 / K
    we1 = 2.0 * np.cos(2 * np.pi * (128 + pc) * e / 512) / K
    wev = np.zeros((128, 2, 260))
    wev[:, 0, 0:NE] = we0
    wev[:, 1, 0:NE] = we1
    d_odd = 2 * np.arange(NO)[None, :] + 1
    wod = np.zeros((128, 2, NO))
    wod[:, 0] = 2.0 * np.cos(2 * np.pi * pc * d_odd / K) / K
    wod[0, 0] = 1.0 / K
    wod[:, 1] = 2.0 * np.cos(2 * np.pi * (128 + pc) * d_odd / K) / K
    # lane-0 correction matrices (only row 0 nonzero; applied as full
    # 128-partition matmuls against the pa1 / sq6 tiles, whose lane 0
    # holds Pf256 / Pf512): [0:260) even wpf - we0row0, [260:520) odd
    # +1/K, [520:780) odd -2/K
    wcor = np.zeros((128, 780))
    wcor[0, 0:NE] = 2.0 * ((-1.0) ** np.arange(NE)) / K - 1.0 / K
    wcor[0, 260 : 260 + NO] = 1.0 / K
    wcor[0, 520 : 520 + NO] = -2.0 / K
    wev = wev.astype(np.float32).astype(NP_BF16)
    wod = wod.astype(np.float32).astype(NP_BF16)
    wcor = wcor.astype(np.float32).astype(NP_BF16)

    ident = np.eye(128, dtype=np.float32).astype(NP_BF16)
    negid = (-np.eye(128, dtype=np.float32)).astype(NP_BF16)

    ntl = 16 - HW_TILES
    gidx = np.zeros((ntl, 128), dtype=np.int32)
    for j, jt in enumerate(range(HW_TILES, 16)):
        b = jt // 8
        h0 = _tile_h0(jt)
        for hh in range(16):
            for c in range(C_DIM):
                gidx[j, hh * 8 + c] = (
                    b * C_DIM * L_DIM + c * L_DIM + POS[h0 + hh]
                )
    gidx_t = np.ascontiguousarray(gidx.T)
    return {"w1e": w1e, "w1o": w1o, "wev": wev, "wod": wod, "wcor": wcor,
            "ident": ident, "negid": negid, "gidx": gidx_t}


def build_program():
    nc = bass.Bass("TRN2", target_bir_lowering=False, debug=False,
                   num_swdge_queues=4)
    x = nc.dram_tensor("x", [B_LOC, C_DIM, L_DIM], BF16,
                       kind="ExternalInput").ap()
    w1e = nc.dram_tensor("w1e", [128, 4, 640], BF16,
                         kind="ExternalInput").ap()
    w1o = nc.dram_tensor("w1o", [128, 4, 512], BF16,
                         kind="ExternalInput").ap()
    wev = nc.dram_tensor("wev", [128, 2, 260], BF16,
                         kind="ExternalInput").ap()
    wod = nc.dram_tensor("wod", [128, 2, 256], BF16,
                         kind="ExternalInput").ap()
    wcor = nc.dram_tensor("wcor", [128, 780], BF16,
                          kind="ExternalInput").ap()
    ident = nc.dram_tensor("ident", [128, 128], BF16,
                           kind="ExternalInput").ap()
    negid = nc.dram_tensor("negid", [128, 128], BF16,
                           kind="ExternalInput").ap()
    gidx = nc.dram_tensor("gidx", [128, 16 - HW_TILES], I32,
                          kind="ExternalInput").ap()
    y = nc.dram_tensor(
        "y", [B_LOC, H_DIM, C_DIM, NOUT], BF16, kind="ExternalOutput"
    ).ap()

    with tile.TileContext(nc) as tc:
        with (
            tc.tile_pool(name="singles", bufs=1) as singles,
            tc.tile_pool(name="gather", bufs=16) as gpool,
            tc.tile_pool(name="grev", bufs=16) as rpool,
            tc.tile_pool(name="wt", bufs=2) as wtpool,
            tc.tile_pool(name="sq", bufs=2) as sqpool,
            tc.tile_pool(name="pa", bufs=2) as papool,
            tc.tile_pool(name="pp", bufs=2) as pppool,
            tc.tile_pool(name="yy", bufs=4) as ypool,
            tc.tile_pool(name="small", bufs=48) as smallpool,
            tc.tile_pool(name="tp_ps", bufs=2, space="PSUM") as tp_ps_pool,
            tc.tile_pool(name="mm1_ps", bufs=2, space="PSUM") as mm1_ps_pool,
            tc.tile_pool(name="mm2_ps", bufs=2, space="PSUM") as mm2_ps_pool,
        ):
            id_sb = singles.tile([128, 128], BF16)
            nc.sync.dma_start(out=id_sb, in_=ident)
            nid_sb = singles.tile([128, 128], BF16)
            nc.sync.dma_start(out=nid_sb, in_=negid)

            gts = [None] * 16
            for _jt in range(16):
                gts[_jt] = gpool.tile([128, 1026], BF16, tag="gt",
                                      name=f"gt{_jt}")

            def emit_gather_runs(jt, eng):
                gt = gts[jt]
                b = jt // 8
                h0 = _tile_h0(jt)
                for (s, r, delta) in _tile_runs(jt):
                    src = bass.AP(
                        tensor=x.tensor,
                        offset=b * C_DIM * L_DIM + POS[h0 + s],
                        ap=[[delta, r], [L_DIM, C_DIM], [1, K_DIM]],
                    )
                    dst = bass.AP(
                        tensor=gt.tensor,
                        offset=gt.offset + 8 * s * 1026,
                        ap=[[1026, 8 * r], [1, K_DIM]],
                    )
                    eng.dma_start(out=dst, in_=src)

            # HWDGE head gathers split across both rings (3 triggers
            # each); the rest ride SWDGE while these cover its preamble
            for _jt in (0, 2):
                emit_gather_runs(_jt, nc.sync)
            for _jt in (1, 3):
                emit_gather_runs(_jt, nc.scalar)

            # weights on the SP ring (one large-descriptor DMA each)
            w1e_sb = singles.tile([128, 4, 640], BF16)
            nc.sync.dma_start(out=w1e_sb, in_=w1e)
            w1o_sb = singles.tile([128, 4, 512], BF16)
            nc.sync.dma_start(out=w1o_sb, in_=w1o)
            wev_sb = singles.tile([128, 2, 260], BF16)
            nc.sync.dma_start(out=wev_sb, in_=wev)
            wod_sb = singles.tile([128, 2, 256], BF16)
            nc.sync.dma_start(out=wod_sb, in_=wod)
            wcor_sb = singles.tile([128, 780], BF16)
            nc.sync.dma_start(out=wcor_sb, in_=wcor)
            gidx_sb = singles.tile([128, 16 - HW_TILES], I32)
            nc.sync.dma_start(out=gidx_sb, in_=gidx)

            # SWDGE gathers for tiles 10..15
            x_flat = x.rearrange("b c l -> (b c) l")
            for jt in range(HW_TILES, 16):
                gt = gts[jt]
                j = jt - HW_TILES
                gd = nc.gpsimd.indirect_dma_start(
                    out=gt[:, 0:K_DIM],
                    out_offset=None,
                    in_=x_flat,
                    in_offset=bass.IndirectOffsetOnAxis(
                        ap=gidx_sb[:, j : j + 1], axis=1
                    ),
                )
                qi = j % 4
                if qi:
                    gd.ins.queue = f"qPoolDynamic{qi}"

            # PE warmup: keep the HAM clock-gate fed until real transposes
            for w in range(13):
                tpw = tp_ps_pool.tile([128, 512], BF16, tag="tp")
                for i in range(4):
                    nc.tensor.transpose(
                        tpw[:, 128 * i : 128 * (i + 1)], id_sb, id_sb
                    )
            for w in range(6):
                tpw = tp_ps_pool.tile([128, 512], BF16, tag="tp")
                for i in range(8):
                    nc.tensor.transpose(
                        tpw[:, 64 * i : 64 * (i + 1)], id_sb, id_sb[:, 0:64]
                    )

            inv2s = [None] * 16
            grevs = [None] * 16

            def tile_prep(jt, rev_eng):
                gt = gts[jt]
                fm = smallpool.tile([128, 1], F32, tag="fm")
                nc.vector.reduce_max(
                    out=fm, in_=gt[:, 0:K_DIM],
                    axis=mybir.AxisListType.X,
                    apply_absolute_value=True,
                )
                inv = smallpool.tile([128, 1], F32, tag="inv")
                nc.vector.reciprocal(out=inv, in_=fm)
                inv2 = smallpool.tile([128, 1], F32, tag="inv2")
                nc.vector.tensor_mul(inv2, inv, inv)
                inv2s[jt] = inv2
                # k=0 slot dies (hann[0]=0) -> zero it so the E fold puts
                # g[512] there (via grev[0] = col 1024); O slot 0 = -g512
                # (zero-weighted)
                nc.vector.memset(gt[:, 0:1], 0)
                nc.vector.tensor_copy(gt[:, 1024:1025], gt[:, 512:513])
                gr = rpool.tile([128, 512], BF16, tag="gr")
                grevs[jt] = gr
                rev = bass.AP(
                    tensor=gt.tensor,
                    offset=gt.offset + 1024,
                    ap=[list(gt.ap[0]), [-1, 512]],
                )
                eng = nc.vector if rev_eng == "v" else nc.gpsimd
                eng.tensor_copy(gr, rev)

            def mm1_q(wt_t, sq, q, rows):
                mq = mm1_ps_pool.tile([128, 512], F32, tag="mm1")
                if q < 4:
                    ops = [(w1e_sb, 128 * q, t, t) for t in range(4)]
                elif q != 6:
                    ops = [(w1o_sb, 128 * (q - 4), t, 4 + t)
                           for t in range(4)]
                else:
                    ops = [(w1o_sb, 256, t, 4 + t) for t in range(4)]
                    ops += [(w1e_sb, 512, t, t) for t in range(4)]
                n = len(ops)
                for i, (wsb, c0, t, slot) in enumerate(ops):
                    nc.tensor.matmul(
                        mq[:, 0:rows],
                        wsb[:, t, c0 : c0 + 128],
                        wt_t[:, slot, :],
                        start=(i == 0),
                        stop=(i == n - 1),
                    )
                nc.scalar.square(sq[:, q, :], mq[:, 0:rows])

            def y_out(yp, jt):
                ysb = ypool.tile([128, 516], BF16, tag="y")
                nc.scalar.mul(ysb[:, 0:NOUT], yp[:, 0:NOUT], inv2s[jt])
                b = jt // 8
                hs = _tile_h0(jt)
                dst = y[b, hs : hs + 16].rearrange("h c n -> (h c) n")
                nc.sync.dma_start(out=dst, in_=ysb[:, 0:NOUT])

            def mm2_head(yp, pp, pa, sq, r0):
                # ONE accumulation group per yp tile: start only on the
                # first matmul, stop only on the last (interleaved
                # start/stop groups within one PSUM bank corrupt each
                # other, measured). The odd half's first writer hits
                # virgin has_written=0 elements, so start=False still
                # overwrites there. skip_group_check: bass pairs
                # start/stop per column-range, the hardware per bank.
                nc.tensor.matmul(
                    yp[:, 0:NE], pp[:, 0, r0 : r0 + 128], wev_sb[:, 0, 0:NE],
                    start=True, stop=False, skip_group_check=True,
                )
                nc.tensor.matmul(
                    yp[:, 0:NE], pa[:, 1, r0 : r0 + 128],
                    wcor_sb[:, 0:NE], start=False, stop=False,
                    skip_group_check=True,
                )
                nc.tensor.matmul(
                    yp[:, NE : NE + 255], pp[:, 1, r0 : r0 + 128],
                    wod_sb[:, 0, 0:255], start=False, stop=False,
                    skip_group_check=True,
                )
                nc.tensor.matmul(
                    yp[:, NE : NE + 255], pa[:, 1, r0 : r0 + 128],
                    wcor_sb[:, 260 : 260 + 255], start=False, stop=False,
                    skip_group_check=True,
                )
                nc.tensor.matmul(
                    yp[:, NE : NE + 255], sq[:, 6, r0 : r0 + 128],
                    wcor_sb[:, 520 : 520 + 255], start=False, stop=False,
                    skip_group_check=True,
                )

            def mm2_tail2(yp, pp, pa, sq, r0):
                nc.tensor.matmul(
                    yp[:, 0:NE], pp[:, 2, r0 : r0 + 128], wev_sb[:, 1, 0:NE],
                    start=False, stop=False, skip_group_check=True,
                )
                nc.tensor.matmul(
                    yp[:, NE : NE + 255], pp[:, 3, r0 : r0 + 128],
                    wod_sb[:, 1, 0:255], start=False, stop=True,
                    skip_group_check=True,
                )
                # odd e=255 (device col 512) lives in the next PSUM bank;
                # it gets its own properly-bracketed 1-col group
                nc.tensor.matmul(
                    yp[:, 512:513], pp[:, 1, r0 : r0 + 128],
                    wod_sb[:, 0, 255:256], start=True, stop=False,
                )
                nc.tensor.matmul(
                    yp[:, 512:513], pa[:, 1, r0 : r0 + 128],
                    wcor_sb[:, 515:516], start=False, stop=False,
                )
                nc.tensor.matmul(
                    yp[:, 512:513], sq[:, 6, r0 : r0 + 128],
                    wcor_sb[:, 775:776], start=False, stop=False,
                )
                nc.tensor.matmul(
                    yp[:, 512:513], pp[:, 3, r0 : r0 + 128],
                    wod_sb[:, 1, 255:256], start=False, stop=True,
                )

            def mm2_rt(tile0, pp, pa, sq, rt):
                jt = tile0 + rt
                yp = mm2_ps_pool.tile([128, 516], F32, tag="mm2")
                r0 = 128 * rt
                mm2_head(yp, pp, pa, sq, r0)
                mm2_tail2(yp, pp, pa, sq, r0)
                y_out(yp, jt)

            GROUPS = [(0, 2), (2, 2), (4, 4), (8, 4), (12, 2), (14, 2)]
            PREP = [(2, 3), (4, 5, 6, 7), (8, 9, 10, 11), (12, 13, 14, 15),
                    (), ()]
            REV_ENG = ["v"] * 10 + ["g"] * 6

            prev = None            # (tile0, pp, pa, sq, ntiles)
            tail_yps = []
            CHUNK_ORDER = (0, 6, 2, 4, 1, 5, 3, 7)
            deferred = []

            def emit_deferred():
                if deferred:
                    djt, (rs, rr, dd) = deferred.pop(0)
                    src = bass.AP(
                        tensor=x.tensor,
                        offset=(djt // 8) * C_DIM * L_DIM
                        + POS[_tile_h0(djt) + rs],
                        ap=[[dd, rr], [L_DIM, C_DIM], [1, K_DIM]],
                    )
                    dst = bass.AP(
                        tensor=gts[djt].tensor,
                        offset=gts[djt].offset + 8 * rs * 1026,
                        ap=[[1026, 8 * rr], [1, K_DIM]],
                    )
                    nc.scalar.dma_start(out=dst, in_=src)

            tile_prep(0, REV_ENG[0])
            tile_prep(1, REV_ENG[1])

            for gi, (tile0, nt) in enumerate(GROUPS):
                last = gi == len(GROUPS) - 1
                rows = 128 * nt
                wt_t = wtpool.tile([128, 8, 512], BF16, tag="wt")
                wt_t = wt_t[:, :, 0:rows]
                sq = sqpool.tile([128, 8, 512], BF16, tag="sq")
                sq = sq[:, :, 0:rows]
                pa = papool.tile([128, 4, 512], BF16, tag="pa")
                pa = pa[:, :, 0:rows]
                pp = pppool.tile([128, 4, 512], BF16, tag="pp")
                pp = pp[:, :, 0:rows]
                prt = 0 if prev is None else prev[4]
                # transposes: each sample chunk is a pair of accumulating
                # REGULAR matmuls against +/-identity (the xbar transpose
                # path ignores PSUM accumulation, measured) -- fwd gather
                # block + grev block realize the E/O fold on the PE, summed
                # in f32 PSUM, then one cast-copy per chunk to bf16 SBUF.
                for slot in range(8):
                    tp = tp_ps_pool.tile([128, 512], F32, tag="tp")
                    blk = slot if slot < 4 else slot - 4
                    rid = id_sb if slot < 4 else nid_sb
                    for i in range(nt):
                        o = 128 * i
                        nc.tensor.matmul(
                            tp[:, o : o + 128],
                            gts[tile0 + i][:, 128 * blk : 128 * blk + 128],
                            id_sb, start=True, stop=False,
                        )
                        nc.tensor.matmul(
                            tp[:, o : o + 128],
                            grevs[tile0 + i][:, 128 * blk : 128 * blk + 128],
                            rid, start=False, stop=True,
                        )
                    if slot % 2 == 0:
                        nc.vector.tensor_copy(wt_t[:, slot, :],
                                              tp[:, 0:rows])
                    else:
                        nc.scalar.copy(out=wt_t[:, slot, :],
                                       in_=tp[:, 0:rows])
                    if last:
                        if slot == 2 and prt > 0:
                            mm2_rt(prev[0], prev[1], prev[2], prev[3], 0)
                        if slot == 5 and prt > 0:
                            mm2_rt(prev[0], prev[1], prev[2], prev[3], 1)
                    elif slot == 5 and prt > 0:
                        mm2_rt(prev[0], prev[1], prev[2], prev[3], 0)
                slist = PREP[gi]
                for step, q in enumerate(CHUNK_ORDER):
                    mm1_q(wt_t, sq, q, rows)
                    if gi < 2:
                        emit_deferred()
                    if step < len(slist):
                        tile_prep(slist[step], REV_ENG[slist[step]])
                    if step == 1:
                        nc.vector.tensor_add(pa[:, 0, :], sq[:, 0, :],
                                             sq[:, 6, :])
                    elif step == 3:
                        nc.vector.tensor_add(pa[:, 1, :], sq[:, 2, :],
                                             sq[:, 4, :])
                        nc.vector.tensor_add(pp[:, 0, :], pa[:, 0, :],
                                             pa[:, 1, :])
                        nc.vector.tensor_sub(pp[:, 1, :], pa[:, 0, :],
                                             pa[:, 1, :])
                    elif step == 5:
                        nc.vector.tensor_add(pa[:, 2, :], sq[:, 1, :],
                                             sq[:, 5, :])
                    elif step == 7:
                        nc.vector.tensor_add(pa[:, 3, :], sq[:, 3, :],
                                             sq[:, 7, :])
                        nc.vector.tensor_add(pp[:, 2, :], pa[:, 2, :],
                                             pa[:, 3, :])
                        nc.vector.tensor_sub(pp[:, 3, :], pa[:, 2, :],
                                             pa[:, 3, :])
                    if step == 1 and prt > 2:
                        mm2_rt(prev[0], prev[1], prev[2], prev[3], 1)
                    if step == 3 and prt > 2:
                        mm2_rt(prev[0], prev[1], prev[2], prev[3], 2)
                    if step == 5 and prt > 2:
                        mm2_rt(prev[0], prev[1], prev[2], prev[3], 3)
                    if step == 3 and not last and 0 < prt <= 2:
                        mm2_rt(prev[0], prev[1], prev[2], prev[3], 1)
                    if last:
                        if step == 3:
                            tail_yps = [
                                mm2_ps_pool.tile(
                                    [128, 516], F32, tag="mm2",
                                    name=f"typ{_rt}",
                                )
                                for _rt in range(nt)
                            ]
                            for rt, yp in enumerate(tail_yps):
                                mm2_head(yp, pp, pa, sq, 128 * rt)
                        if step == 7:
                            for rt, yp in enumerate(tail_yps):
                                mm2_tail2(yp, pp, pa, sq, 128 * rt)
                                y_out(yp, tile0 + rt)
                prev = (tile0, pp, pa, sq, nt)
    return nc


def get_program():
    if "nc" not in _prog_cache:
        _prog_cache["nc"] = build_program()
        _prog_cache["consts"] = make_constants()
    return _prog_cache["nc"], _prog_cache["consts"]


def kernel(X, kernel_size=None, out_channels=None, _trace=False):
    X = np.ascontiguousarray(
        np.asarray(X, dtype=np.float32).astype(NP_BF16)
    )
    assert X.shape == (B_FULL, C_DIM, L_DIM)
    nc, consts = get_program()
    in_maps = []
    for c in range(N_CORES):
        m = {"x": X[c * B_LOC : (c + 1) * B_LOC]}
        m.update(consts)
        in_maps.append(m)
    res = run_bass_kernel_spmd(
        nc, in_maps, core_ids=list(range(N_CORES)), trace=_trace
    )
    raw = np.concatenate(
        [np.asarray(r["y"]).astype(np.float32) for r in res.results], axis=0
    )  # (B, H, C, 513): cols 0:257 = acov[0::2], 257:513 = acov[1::2]
    acov = np.empty_like(raw)
    acov[..., 0::2] = raw[..., 0:NE]
    acov[..., 1::2] = raw[..., NE:NOUT]
    out = np.concatenate(
        [acov[..., 512:0:-1], acov[..., 0:512]], axis=-1
    )
    if _trace:
        return out, res
    return out
